# revision 1
# baseline (speedup 1.0000x reference)
"""Trainium2 Bass kernel for a 13-layer causal dilated conv stack with gating.

Model (per reference):
    Wx_f = 13 causal dilated convs (K=2, dilation 2^i) over x with Wf
    Wx_g = same with Wg
    out  = tanh(Wx_f + h@Vf) * sigmoid(Wx_g + h@Vg)

Shapes: x (16, 8192, 64) f32, h (16, 64), Wf/Wg (13, 2, 64, 64), Vf/Vg (64, 64).

Strategy:
  - Data-parallel over batch: 2 batch elements per core on 8 cores, no
    collectives.
  - On-chip layout [128 partitions = (b*64 + c), T free]: both local batch
    elements' channels stacked on the partition axis. The host pre-transposes
    x into this channel-major layout (and transposes the output back), so the
    device does no layout changes at all.
  - Each conv layer is, per 512-token tile, two accumulating PE matmuls (one
    per tap) with block-diagonal kron(I2, W[tap]) stationary weights -> full
    128-wide PE utilization.
  - Causality: activation buffers carry a 256-column zero margin covering
    dilations < 512; for d >= 512 the boundary is tile-aligned and the tap-0
    matmul is simply skipped on the first d/512 tiles.
  - Full fp16 datapath (x, weights, inter-layer activations): full-rate PE
    like fp32r, but halves LDWEIGHTS SBUF traffic (the per-matmul weight
    reload is fully hidden at ~216ns matmul spacing vs ~227ns for f32r),
    halves input DMA, and keeps 10-bit mantissas so the 26-layer stack
    stays at ~4e-3 relative error (bf16's 8-bit mantissa fails the gate).
    PSUM accumulation is f32 throughout; mixing 16/32-bit matmul operands
    is not supported by the backend, so weights and activations must both
    be fp16.
  - h@V bias is computed on-device with kron(I2, V) and fused into the
    tanh/sigmoid activations via the ScalarE bias port.
  - PSUM->SBUF drain copies alternate DVE/DVE/ACT so neither engine
    bottlenecks the PE (scratch stays DVE-only for the output DMAs).
  - Startup: DMA issue order is consts -> f0 weights -> a small (512 col)
    first x chunk -> rest of the weight head -> remaining x chunks -> weight
    tail, so the first conv matmul is gated only by a 256KB transfer instead
    of the full first-chunk + weight-head serialization.
  - Endgame: the last two g layers are issued interleaved per tile, so the
    16 sigmoid+mul epilogues overlap the matmul stream instead of chaining
    after it. tanh/sigmoid/mul run in bf16 (2x DVE rate) and the output is
    DMA'd out as bf16 (half the bytes) in progressively smaller chunks, then
    widened to f32 on the host. bf16 rounding enters only in this final
    nonlinear stage, adding ~1e-3 relative error.
  - PE matmuls and HWDGE DMA descriptors only support a single sync wait and
    Tile's wait pass is not transitively minimal. The kernel therefore (a)
    warms the PE's vector clock with one tiny matmul per input-DMA lane so
    real matmuls never re-wait DMA lanes, and (b) keeps each input / output
    DMA stream on its own HWDGE lane with single-engine dependencies.
"""

import sys

import numpy as np

for _p in ("/opt/trn_rl_repo",):
    if _p not in sys.path:
        sys.path.append(_p)

B, T, C = 16, 8192, 64
K = 2
NUM_LAYERS = 13
N_CORES = 8
BPC = B // N_CORES          # batch elements per core
P = 2 * C                   # partitions used: (b, c) pairs
NTAP_TILE = 512             # tokens per matmul tile
NT = T // NTAP_TILE         # matmul tiles per layer
MARGIN = 256                # causal zero margin (covers dilations < 512)
NW = 2 * NUM_LAYERS * K     # packed conv weight count
W_HEAD = 8                  # weight tiles in the head DMA (first 4 layers run)
W_F0 = 2                    # f-l0's two weight tiles, DMA'd before x chunk 0

# x input chunks: small first chunk so layer-0 compute starts ASAP
XEDGE = [0, 512, 1536, 2560, 3584, 4608, 5632, 6656, 8192]
# output chunks (bf16), big early / small late so the final transfer is tiny
QEDGE = [0, 2048, 4096, 6144, 7168, 7680, 8192]
NQ = len(QEDGE) - 1

# layer execution schedule (branch, layer), chosen so BOTH branches' layer 0
# run during the x-chunk-paced startup (g-l0 parks its output in scratch,
# which is otherwise idle until f-l12), and weights are packed in this order
# so the head DMA covers exactly the first layers. The last two entries
# (g-l11, g-l12) are issued interleaved per tile by the tail loop.
SCHED = ([("f", 0), ("g", 0)] + [("f", l) for l in range(1, 12)] +
         [("g", 1), ("f", 12)] + [("g", l) for l in range(2, 13)])

_PROGRAM_CACHE = {}


def fp32r_bits(a):
    """Round f32 to fp32r (11-bit mantissa, RNE), low 12 bits zeroed."""
    u = np.ascontiguousarray(a, dtype=np.float32).view(np.uint32)
    keep = u >> np.uint32(12)
    low = u & np.uint32(0xFFF)
    rup = (low > 0x800) | ((low == 0x800) & ((keep & np.uint32(1)) == 1))
    return ((keep + rup.astype(np.uint32)) << np.uint32(12)).view(np.float32)


def _build_program():
    import concourse.bacc as bacc
    import concourse.tile as tile
    from concourse import mybir

    f32 = mybir.dt.float32
    f32r = mybir.dt.float32r
    bf16 = mybir.dt.bfloat16
    fp16 = mybir.dt.float16
    AF = mybir.ActivationFunctionType

    # Bacc (not bare Bass): its compile() legalizes multi-wait instructions
    # into event-semaphore chains (TRN2 allows 1 wait per instruction).
    nc = bacc.Bacc("TRN2", target_bir_lowering=False, debug=False)

    consts = nc.dram_tensor("consts", [P, 2 * P + 1], f32,
                            kind="ExternalInput").ap()
    xin = nc.dram_tensor("xr", [P, T], fp16, kind="ExternalInput").ap()
    wconv = nc.dram_tensor("wr", [P, NW * P], fp16, kind="ExternalInput").ap()
    outs = [nc.dram_tensor(f"out_q{q}", [P, QEDGE[q + 1] - QEDGE[q]], fp16,
                           kind="ExternalOutput").ap() for q in range(NQ)]

    with tile.TileContext(nc) as tc:
        with (
            tc.tile_pool(name="persist", bufs=1) as persist,
            tc.tile_pool(name="epool", bufs=4) as epool,
            tc.tile_pool(name="mpsum", bufs=8, space="PSUM") as mpsum,
        ):
            # ---- persistent buffers ---------------------------------------
            scratch = persist.tile([P, MARGIN + T], fp16, name="scratch",
                                   tag="scratch")
            call = persist.tile([P, 2 * P + 1], f32, name="call", tag="call")
            x0 = persist.tile([P, MARGIN + T], fp16, name="x0", tag="x0")
            bufA = persist.tile([P, MARGIN + T], fp16, name="bufA", tag="bufA")
            bufB = persist.tile([P, MARGIN + T], fp16, name="bufB", tag="bufB")
            wall = persist.tile([P, NW * P], fp16, name="wall", tag="wall")
            for buf in (x0, bufA, bufB, scratch):
                nc.vector.memset(buf[:, 0:MARGIN], 0.0)
            # dedicated bf16 buffers for the final nonlinear stage (the BIR
            # verifier rejects non-f32r-typed writes into any location an
            # f32r matmul reads, so these cannot alias x0/scratch)
            tanh16 = persist.tile([P, T], fp16, name="tanh16", tag="tanh16")
            out16 = persist.tile([P, T], fp16, name="out16", tag="out16")

            # ---- input DMAs, ordered by first consumption -----------------
            # consts -> f-l0 weights -> small x chunk 0 -> rest of weight
            # head -> remaining x chunks -> weight tail, so layer-0 compute
            # starts as early as the DMA wake-up latency allows.
            nc.sync.dma_start(
                out=x0[:, MARGIN + XEDGE[0]:MARGIN + XEDGE[1]],
                in_=xin[:, XEDGE[0]:XEDGE[1]])
            nc.sync.dma_start(out=wall[:, 0:W_F0 * P],
                              in_=wconv[:, 0:W_F0 * P])
            nc.sync.dma_start(out=call, in_=consts)
            nc.sync.dma_start(out=wall[:, W_F0 * P:W_HEAD * P],
                              in_=wconv[:, W_F0 * P:W_HEAD * P])
            for xc in range(1, len(XEDGE) - 1):
                nc.sync.dma_start(
                    out=x0[:, MARGIN + XEDGE[xc]:MARGIN + XEDGE[xc + 1]],
                    in_=xin[:, XEDGE[xc]:XEDGE[xc + 1]])
            nc.sync.dma_start(out=wall[:, W_HEAD * P:],
                              in_=wconv[:, W_HEAD * P:])

            # ---- PE lane warm-ups + h @ V biases --------------------------
            # single-shot matmuls into disjoint columns of one PSUM tile:
            # three consume the x / w-head0 / w-head DMA lanes (results
            # unused), then the two bias projections (their lane comes via
            # their own operands). lives in the conv-psum ring: only needed
            # in the first few us, so it must not pin a PSUM bank for the
            # whole kernel
            bias_ps = mpsum.tile([P, 8], f32, name="bias_ps", tag="mp")
            x0w = x0[:, MARGIN:MARGIN + 1]
            ww0 = wall[:, 0:1]
            ww1 = wall[:, W_F0 * P:W_F0 * P + 1]
            nc.tensor.matmul(bias_ps[0:1, 4:5], lhsT=x0w, rhs=x0w,
                             start=True, stop=True)
            nc.tensor.matmul(bias_ps[0:1, 5:6], lhsT=ww0, rhs=ww0,
                             start=True, stop=True)
            nc.tensor.matmul(bias_ps[0:1, 6:7], lhsT=ww1, rhs=ww1,
                             start=True, stop=True)
            h_t = call[:, 2 * P:2 * P + 1]
            bias = []
            for i in range(2):
                nc.tensor.matmul(bias_ps[:, i:i + 1],
                                 lhsT=call[:, i * P:(i + 1) * P],
                                 rhs=h_t, start=True, stop=True)
                bias_sb = persist.tile([P, 1], f32, name=f"bias{i}",
                                       tag=f"bias{i}")
                nc.vector.tensor_copy(bias_sb, bias_ps[:, i:i + 1])
                bias.append(bias_sb)

            # ---- conv stacks ----------------------------------------------
            drain_rr = [0]

            def conv_tile(src, layer, br, j, dst=None, fuse=None):
                d = 2 ** layer
                base = 2 * SCHED.index((br, layer))
                w0r = wall[:, base * P:(base + 1) * P]
                w1r = wall[:, (base + 1) * P:(base + 2) * P]
                t0 = j * NTAP_TILE
                # d < 512 boundary reads dip into the zero margin;
                # d >= 512 boundaries are tile-aligned -> tap 0 skipped.
                has_tap0 = t0 + NTAP_TILE > d
                ps = mpsum.tile([P, NTAP_TILE], f32,
                                name=f"ps_{br}{layer}_{j}", tag="mp")
                nc.tensor.matmul(
                    ps, lhsT=w1r,
                    rhs=src[:, MARGIN + t0:MARGIN + t0 + NTAP_TILE],
                    start=True, stop=not has_tap0)
                if has_tap0:
                    o0 = MARGIN + t0 - d
                    nc.tensor.matmul(
                        ps, lhsT=w0r, rhs=src[:, o0:o0 + NTAP_TILE],
                        start=False, stop=True)
                if fuse is None:
                    dslice = dst[:, MARGIN + t0:MARGIN + t0 + NTAP_TILE]
                    # alternate drains DVE/ACT so neither engine gates PE
                    # (GpSimd cannot read PSUM, so no 3-way rotation)
                    if drain_rr[0] % 2 == 1:
                        nc.scalar.copy(dslice, ps)
                    else:
                        nc.vector.tensor_copy(dslice, ps)
                    drain_rr[0] += 1
                else:
                    fuse(j, ps)

            def conv_layer(src, layer, br, dst=None, fuse=None):
                for j in range(NT):
                    conv_tile(src, layer, br, j, dst=dst, fuse=fuse)

            # f-l12's drain IS the tanh (ScalarE, fused bias), in bf16,
            # parked in scratch's head bytes for the g-l12 epilogue.
            def tanh_drain(j, ps_f):
                t0 = j * NTAP_TILE
                nc.scalar.activation(tanh16[:, t0:t0 + NTAP_TILE], ps_f,
                                     AF.Tanh, bias=bias[0])

            # g-l12 fused with the gating epilogue, all in bf16
            def epilogue(j, ps_g):
                t0 = j * NTAP_TILE
                sig = epool.tile([P, NTAP_TILE], fp16, name=f"sig{j}",
                                 tag="sig")
                nc.scalar.activation(sig, ps_g, AF.Sigmoid, bias=bias[1])
                nc.vector.tensor_mul(out16[:, t0:t0 + NTAP_TILE],
                                     tanh16[:, t0:t0 + NTAP_TILE], sig)

            # buffer rotation per SCHED:
            #   f: x0 -> A -> B -> A ... (f-l12 reads B, tanh -> scratch)
            #   g: x0 -> scratch -> A -> B ... (g-l12 reads A, epilogue)
            cur = {"f": x0, "g": x0}
            for br, layer in SCHED[:-2]:
                if (br, layer) == ("f", 12):
                    conv_layer(cur["f"], layer, br, fuse=tanh_drain)
                else:
                    if br == "f":
                        dst = bufA if layer % 2 == 0 else bufB
                    else:
                        dst = scratch if layer == 0 else \
                            (bufA if layer % 2 == 1 else bufB)
                    conv_layer(cur[br], layer, br, dst=dst)
                    cur[br] = dst

            # ---- tail: g-l11 and g-l12 interleaved per tile ---------------
            # g-l12 tile j only needs g-l11 tiles <= j (tap 0 reads 8 tiles
            # back), so a 2-tile lag spreads the 16 sigmoid+mul epilogues
            # across the final matmul stream instead of chaining after it.
            # Output chunks are DMA'd as soon as their last tile's mul is
            # issued (channel-major bf16; host widens and restores [b,t,c]).
            src11 = cur["g"]
            nxt_q = [0]

            def flush_outputs(jj):
                while (nxt_q[0] < NQ
                       and QEDGE[nxt_q[0] + 1] <= (jj + 1) * NTAP_TILE):
                    q = nxt_q[0]
                    nc.sync.dma_start(
                        out=outs[q], in_=out16[:, QEDGE[q]:QEDGE[q + 1]])
                    nxt_q[0] += 1

            for j in range(NT):
                conv_tile(src11, 11, "g", j, dst=bufA)
                if j >= 2:
                    conv_tile(bufA, 12, "g", j - 2, fuse=epilogue)
                    flush_outputs(j - 2)
            for j in (NT - 2, NT - 1):
                conv_tile(bufA, 12, "g", j, fuse=epilogue)
                flush_outputs(j)

    nc.compile()
    return nc


def get_program():
    if "nc" not in _PROGRAM_CACHE:
        _PROGRAM_CACHE["nc"] = _build_program()
    return _PROGRAM_CACHE["nc"]


def make_in_maps(x, h, Wf, Wg, Vf, Vg):
    x = np.asarray(x, dtype=np.float32)
    h = np.asarray(h, dtype=np.float32)
    eye2 = np.eye(2, dtype=np.float32)
    # SCHED-ordered [branch, layer, tap] -> kron(I2, W[tap]) as lhsT
    # [K=(b,cin), M=(b,cout)]
    Wn = {"f": np.asarray(Wf, dtype=np.float32),
          "g": np.asarray(Wg, dtype=np.float32)}
    wpack = np.zeros((NW, P, P), dtype=np.float32)
    for pos, (br, layer) in enumerate(SCHED):
        for tap in range(K):
            wpack[2 * pos + tap] = np.kron(eye2, Wn[br][layer, tap])
    # wall[p, i*P + m] = wpack[i, p, m], rounded to fp16 (10-bit mantissa;
    # conv weights are ~0.09 magnitude, well inside fp16 range)
    wcols = wpack.transpose(1, 0, 2).reshape(P, NW * P).astype(np.float16)
    vcat = np.concatenate(
        [np.kron(eye2, np.asarray(V, dtype=np.float32)) for V in (Vf, Vg)],
        axis=1)  # [128, 256]

    in_maps = []
    for core in range(N_CORES):
        sl = slice(core * BPC, (core + 1) * BPC)
        xcm = x[sl].transpose(0, 2, 1).reshape(P, T) \
            .astype(np.float16)  # [(b,c), t]
        consts = np.ascontiguousarray(
            np.concatenate([vcat, h[sl].reshape(P, 1)], axis=1))
        in_maps.append({"consts": consts, "xr": xcm, "wr": wcols})
    return in_maps


def _to_f32(a):
    """16-bit float (fp16, or bf16 in any container dtype) -> f32."""
    a = np.asarray(a)
    if a.dtype in (np.float32, np.float16):
        return a.astype(np.float32)
    u = a.view(np.uint16).astype(np.uint32) << np.uint32(16)
    return u.view(np.float32)


def assemble_output(results):
    full = np.empty((B, T, C), dtype=np.float32)
    for core, r in enumerate(results):
        cm = np.concatenate(
            [_to_f32(r[f"out_q{q}"]) for q in range(NQ)], axis=1)
        full[core * BPC:(core + 1) * BPC] = \
            cm.reshape(BPC, C, T).transpose(0, 2, 1)
    return full


def kernel(x, h, Wf, Wg, Vf, Vg):
    from concourse import bass_utils

    nc = get_program()
    in_maps = make_in_maps(x, h, Wf, Wg, Vf, Vg)
    res = bass_utils.run_bass_kernel_spmd(nc, in_maps,
                                          core_ids=list(range(N_CORES)))
    return assemble_output(res.results)



# revision 7
# speedup vs baseline: 1.0038x; 1.0038x over previous
"""Trainium2 Bass kernel for a 13-layer causal dilated conv stack with gating.

Model (per reference):
    Wx_f = 13 causal dilated convs (K=2, dilation 2^i) over x with Wf
    Wx_g = same with Wg
    out  = tanh(Wx_f + h@Vf) * sigmoid(Wx_g + h@Vg)

Shapes: x (16, 8192, 64) f32, h (16, 64), Wf/Wg (13, 2, 64, 64), Vf/Vg (64, 64).

Strategy (v2: fused layer pairs + Karatsuba):
  - Data-parallel over batch: 2 batch elements per core on 8 cores, no
    collectives. On-chip layout [128 partitions = (b*64 + c), T free], fp16
    datapath, host pre/post transposes (as v1).
  - Layers are fused in PAIRS: layers (2k, 2k+1) compose into a single 4-tap
    conv with dilation d=4^k and host-precomputed product weights
    V0..V3 (y = x@V0 + x(-d)@V1 + x(-2d)@V2 + x(-3d)@V3). This halves the
    number of PSUM->SBUF activation materializations (the v1 co-bottleneck:
    ACT/DVE drains at ~1.35ns/col vs PE 0.42ns/col).
  - Each composite conv uses KARATSUBA over its z^(2d) structure: with
    P = V0 + V1 z^d, Q = V2 + V3 z^d, and D2(t) = x(t) - x(t-2d),
      even cols (t mod 4d < 2d):  y = p - (Q D2),  p = ((P+Q) x)
      odd  cols (t1 = t - 2d even): y = p(t1) + (P D2)(t)
    -> 6 matmul-cols per 2 output cols instead of 8: 25% less PE time.
    The -Q D2 accumulates into its own PSUM tile; y-even = DVE add of the
    two PSUM tiles; then the P D2 matmuls ACCUMULATE into the p PSUM tile
    (after the DVE read) so y-odd is a plain ACT copy. No p parking.
  - D2 difference tensors are produced by DVE (fp16 2x) and GpSimd
    alternately, into two ping-pong D buffers so the next pair's D overlaps
    the current pair's matmul stream.
  - Layer 12 (d=4096) stays un-fused (odd layer count): standard 2-matmul
    tiles; its drain IS the tanh/sigmoid epilogue as in v1.
  - Causality: 256-col zero margins cover all margin reads for pairs 0-3
    (reads reach -3d, d<=64). Pair 4 (d=256): D2 tile 0 is a copy; offsets
    are tile-aligned. Pair 5 (d=1024): D2 tiles 0-1 are copies and the
    -1024-offset matmuls are skipped on tiles 0-1. Solo layer 12 skips tap0
    on the first 8 tiles.
  - Startup: f-pair0 and g-pair0 run interleaved per supertile (they share
    the same D2 of x), paced by chunked x DMA; weight DMA is split
    first-needed-first as in v1. PE lane warm-ups + h@V bias as v1.
  - Tail: g-pair5 tiles interleave with solo-g tiles, sigmoid+mul epilogue
    and progressive bf16 output DMA chunks as v1.
"""

import sys

import numpy as np

for _p in ("/opt/trn_rl_repo",):
    if _p not in sys.path:
        sys.path.append(_p)

B, T, C = 16, 8192, 64
K = 2
NUM_LAYERS = 13
N_CORES = 8
BPC = B // N_CORES          # batch elements per core
P = 2 * C                   # partitions used: (b, c) pairs
MARGIN = 256                # causal zero margin
ST = 1024                   # supertile cols (pairs 0-4)
NST = T // ST               # 8
TILE = 512                  # tile cols (pair 5, solo, drains, psum width)
NT = T // TILE              # 16
PAIR_D = [1, 4, 16, 64, 256, 1024]   # first-layer dilation of pair k
NPAIR = 6
SOLO_D = 4096

# schedule: (kind, branch, pair_k). f0+g0 run interleaved at startup.
SCHED = ([("pair", "f", 0), ("pair", "g", 0)]
         + [("pair", "f", k) for k in range(1, 6)] + [("solo", "f", None)]
         + [("pair", "g", k) for k in range(1, 6)] + [("solo", "g", None)])

# weight slot base per entry (6 tiles per pair: A0,A1,Qn0,Qn1,P0,P1;
# 2 per solo: W1,W0)
WBASE = {}
_w = 0
for _e in SCHED:
    WBASE[(_e[0], _e[1], _e[2])] = _w
    _w += 6 if _e[0] == "pair" else 2
NW = _w                     # 76

W_FIRST = 2                 # f-pair0's A0,A1: DMA'd before x chunk 0
W_HEAD = 12                 # f0+g0 weights

# x input chunks (1024-aligned so supertile s gates on chunk s)
XEDGE = [0, 1024, 2048, 3072, 4096, 5120, 6144, 7168, 8192]
# output chunks, big early / small late so the final transfer is tiny
QEDGE = [0, 2048, 4096, 6144, 7168, 7680, 8192]
NQ = len(QEDGE) - 1

_PROGRAM_CACHE = {}


def _build_program():
    import concourse.bacc as bacc
    import concourse.tile as tile
    from concourse import mybir
    from concourse.ap import AP

    f32 = mybir.dt.float32
    fp16 = mybir.dt.float16
    AF = mybir.ActivationFunctionType

    nc = bacc.Bacc("TRN2", target_bir_lowering=False, debug=False)

    consts = nc.dram_tensor("consts", [P, 2 * P + 1], f32,
                            kind="ExternalInput").ap()
    xin = nc.dram_tensor("xr", [P, T], fp16, kind="ExternalInput").ap()
    wconv = nc.dram_tensor("wr", [P, NW * P], fp16, kind="ExternalInput").ap()
    outs = [nc.dram_tensor(f"out_q{q}", [P, QEDGE[q + 1] - QEDGE[q]], fp16,
                           kind="ExternalOutput").ap() for q in range(NQ)]

    with tile.TileContext(nc) as tc:
        with (
            tc.tile_pool(name="persist", bufs=1) as persist,
            tc.tile_pool(name="epool", bufs=4) as epool,
            tc.tile_pool(name="qpool", bufs=3) as qpool,
            tc.tile_pool(name="mpsum", bufs=4, space="PSUM") as mpsum,
            tc.tile_pool(name="p5psum", bufs=4, space="PSUM") as p5psum,
        ):
            # ---- persistent buffers ---------------------------------------
            x0 = persist.tile([P, MARGIN + T], fp16, name="x0", tag="x0")
            bufA = persist.tile([P, MARGIN + T], fp16, name="bufA", tag="bufA")
            bufB = persist.tile([P, MARGIN + T], fp16, name="bufB", tag="bufB")
            scratch = persist.tile([P, MARGIN + T], fp16, name="scratch",
                                   tag="scratch")
            D0 = persist.tile([P, MARGIN + T], fp16, name="D0", tag="D0")
            D1 = persist.tile([P, MARGIN + T], fp16, name="D1", tag="D1")
            call = persist.tile([P, 2 * P + 1], f32, name="call", tag="call")
            wall = persist.tile([P, NW * P], fp16, name="wall", tag="wall")
            tanh16 = persist.tile([P, T], fp16, name="tanh16", tag="tanh16")
            out16 = persist.tile([P, T], fp16, name="out16", tag="out16")
            for buf in (x0, bufA, bufB, scratch, D0, D1):
                nc.vector.memset(buf[:, 0:MARGIN], 0.0)

            # ---- input DMAs, ordered by first consumption -----------------
            nc.sync.dma_start(out=wall[:, 0:W_FIRST * P],
                              in_=wconv[:, 0:W_FIRST * P])
            nc.sync.dma_start(
                out=x0[:, MARGIN + XEDGE[0]:MARGIN + XEDGE[1]],
                in_=xin[:, XEDGE[0]:XEDGE[1]])
            nc.sync.dma_start(out=call, in_=consts)
            nc.sync.dma_start(out=wall[:, W_FIRST * P:W_HEAD * P],
                              in_=wconv[:, W_FIRST * P:W_HEAD * P])
            for xc in range(1, len(XEDGE) - 1):
                nc.sync.dma_start(
                    out=x0[:, MARGIN + XEDGE[xc]:MARGIN + XEDGE[xc + 1]],
                    in_=xin[:, XEDGE[xc]:XEDGE[xc + 1]])
            nc.sync.dma_start(out=wall[:, W_HEAD * P:],
                              in_=wconv[:, W_HEAD * P:])

            # ---- PE lane warm-ups + h @ V biases --------------------------
            bias_ps = mpsum.tile([P, 8], f32, name="bias_ps", tag="mp")
            x0w = x0[:, MARGIN:MARGIN + 1]
            ww0 = wall[:, 0:1]
            ww1 = wall[:, W_FIRST * P:W_FIRST * P + 1]
            ww2 = wall[:, W_HEAD * P:W_HEAD * P + 1]
            nc.tensor.matmul(bias_ps[0:1, 4:5], lhsT=x0w, rhs=x0w,
                             start=True, stop=True)
            nc.tensor.matmul(bias_ps[0:1, 5:6], lhsT=ww0, rhs=ww0,
                             start=True, stop=True)
            nc.tensor.matmul(bias_ps[0:1, 6:7], lhsT=ww1, rhs=ww1,
                             start=True, stop=True)
            nc.tensor.matmul(bias_ps[0:1, 7:8], lhsT=ww2, rhs=ww2,
                             start=True, stop=True)
            h_t = call[:, 2 * P:2 * P + 1]
            bias = []
            for i in range(2):
                nc.tensor.matmul(bias_ps[:, i:i + 1],
                                 lhsT=call[:, i * P:(i + 1) * P],
                                 rhs=h_t, start=True, stop=True)
                bias_sb = persist.tile([P, 1], f32, name=f"bias{i}",
                                       tag=f"bias{i}")
                nc.vector.tensor_copy(bias_sb, bias_ps[:, i:i + 1])
                bias.append(bias_sb)

            # ---- helpers --------------------------------------------------
            def sap(buf, col, blkstride, nblk, blklen):
                """Strided AP: nblk blocks of blklen cols every blkstride."""
                base = buf[:, col:col + 1]
                return AP(base.tensor, base.offset,
                          [list(base.ap[0]), [blkstride, nblk], [1, blklen]])

            def wt(base, i):
                return wall[:, (base + i) * P:(base + i + 1) * P]

            dctr = [0]          # D-emission round robin (buffer choice)
            ectr = [0]          # D engine round robin

            def emit_D(db, src, k, s):
                """D2 for pair k over supertile s into db (alternate engines).
                Pair 4: tile 0 of supertile 0 is a copy. Pair 5: supertiles
                0-1 are copies."""
                twod = 2 * PAIR_D[k]
                c0 = s * ST
                eng = nc.vector if ectr[0] % 2 == 0 else nc.gpsimd
                ectr[0] += 1
                dst = db[:, MARGIN + c0:MARGIN + c0 + ST]
                if twod <= MARGIN:
                    eng.tensor_sub(dst, src[:, MARGIN + c0:MARGIN + c0 + ST],
                                   src[:, MARGIN + c0 - twod:
                                       MARGIN + c0 + ST - twod])
                elif twod == TILE:  # pair 4
                    if s == 0:
                        eng.tensor_copy(db[:, MARGIN:MARGIN + TILE],
                                        src[:, MARGIN:MARGIN + TILE])
                        eng.tensor_sub(
                            db[:, MARGIN + TILE:MARGIN + ST],
                            src[:, MARGIN + TILE:MARGIN + ST],
                            src[:, MARGIN:MARGIN + TILE])
                    else:
                        eng.tensor_sub(dst,
                                       src[:, MARGIN + c0:MARGIN + c0 + ST],
                                       src[:, MARGIN + c0 - twod:
                                           MARGIN + c0 + ST - twod])
                else:  # pair 5, twod = 2048
                    if s < 2:
                        eng.tensor_copy(dst,
                                        src[:, MARGIN + c0:MARGIN + c0 + ST])
                    else:
                        eng.tensor_sub(dst,
                                       src[:, MARGIN + c0:MARGIN + c0 + ST],
                                       src[:, MARGIN + c0 - twod:
                                           MARGIN + c0 + ST - twod])

            drain_rr = [0]

            def pair_pq(br, k, s, src, dst, db, wb):
                """p and q matmuls + y-even for supertile s; returns pp."""
                d = PAIR_D[k]
                twod, fourd = 2 * d, 4 * d
                c0 = s * ST
                nb = ST // fourd

                def ev(buf, off):
                    return sap(buf, MARGIN + c0 - off, fourd, nb, twod)

                pp = mpsum.tile([P, ST // 2], f32, name=f"pp_{br}{k}_{s}",
                                tag="mp")
                pq = mpsum.tile([P, ST // 2], f32, name=f"pq_{br}{k}_{s}",
                                tag="mp")
                nc.tensor.matmul(pp, lhsT=wt(wb, 0), rhs=ev(src, 0),
                                 start=True, stop=False)
                nc.tensor.matmul(pp, lhsT=wt(wb, 1), rhs=ev(src, d),
                                 start=False, stop=True)
                nc.tensor.matmul(pq, lhsT=wt(wb, 2), rhs=ev(db, 0),
                                 start=True, stop=False)
                nc.tensor.matmul(pq, lhsT=wt(wb, 3), rhs=ev(db, d),
                                 start=False, stop=True)
                # verifier forbids dual-PSUM TT: bounce q through SBUF (ACT),
                # then y-even = pp(PSUM) + q16(SBUF) on DVE; pp survives for
                # the r accumulation.
                q16 = qpool.tile([P, ST // 2], fp16, name=f"q16_{br}{k}_{s}",
                                 tag="q16")
                nc.scalar.copy(q16, pq)
                nc.vector.tensor_add(ev(dst, 0), pp, q16)
                return pp

            def pair_r(br, k, s, dst, db, wb, pp):
                """r matmuls accumulate into pp; y-odd copy (ACT)."""
                d = PAIR_D[k]
                twod, fourd = 2 * d, 4 * d
                c0 = s * ST
                nb = ST // fourd

                def ev(buf, off):
                    return sap(buf, MARGIN + c0 - off, fourd, nb, twod)

                nc.tensor.matmul(pp, lhsT=wt(wb, 4), rhs=ev(db, -twod),
                                 start=False, stop=False,
                                 skip_group_check=True)
                nc.tensor.matmul(pp, lhsT=wt(wb, 5), rhs=ev(db, -twod + d),
                                 start=False, stop=True,
                                 skip_group_check=True)
                nc.scalar.copy(ev(dst, -twod), pp)

            def pair5_tile(br, j, src, dst, db, wb, p5live):
                """Pair 5 (d=1024) at 512-col tile granularity."""
                c0 = j * TILE
                sl = lambda buf, off: buf[:, MARGIN + c0 - off:
                                          MARGIN + c0 - off + TILE]
                if j % 8 < 4:       # even-block tile
                    pp = p5psum.tile([P, TILE], f32, name=f"p5_{br}_{j}",
                                     tag="p5")
                    skip = (j % 8) < 2 and j < 8
                    nc.tensor.matmul(pp, lhsT=wt(wb, 0), rhs=sl(src, 0),
                                     start=True, stop=skip)
                    if not skip:
                        nc.tensor.matmul(pp, lhsT=wt(wb, 1),
                                         rhs=sl(src, 1024),
                                         start=False, stop=True)
                    pq = mpsum.tile([P, TILE], f32, name=f"p5q_{br}_{j}",
                                    tag="mp")
                    nc.tensor.matmul(pq, lhsT=wt(wb, 2), rhs=sl(db, 0),
                                     start=True, stop=skip)
                    if not skip:
                        nc.tensor.matmul(pq, lhsT=wt(wb, 3),
                                         rhs=sl(db, 1024),
                                         start=False, stop=True)
                    q16 = qpool.tile([P, TILE], fp16, name=f"q5_{br}_{j}",
                                     tag="q16")
                    nc.scalar.copy(q16, pq)
                    nc.vector.tensor_add(sl(dst, 0), pp, q16)
                    p5live[j] = pp
                else:               # odd-block tile: partner is j-4
                    pp = p5live.pop(j - 4)
                    nc.tensor.matmul(pp, lhsT=wt(wb, 4), rhs=sl(db, 0),
                                     start=False, stop=False,
                                     skip_group_check=True)
                    nc.tensor.matmul(pp, lhsT=wt(wb, 5), rhs=sl(db, 1024),
                                     start=False, stop=True,
                                     skip_group_check=True)
                    nc.scalar.copy(sl(dst, 0), pp)

            def solo_tile(br, j, src, wb, fuse):
                """Solo layer 12 (d=4096): psum -> fuse(j, ps)."""
                c0 = j * TILE
                ps = mpsum.tile([P, TILE], f32, name=f"s12{br}_{j}", tag="mp")
                has0 = c0 >= SOLO_D
                nc.tensor.matmul(ps, lhsT=wt(wb, 0),
                                 rhs=src[:, MARGIN + c0:MARGIN + c0 + TILE],
                                 start=True, stop=not has0)
                if has0:
                    nc.tensor.matmul(
                        ps, lhsT=wt(wb, 1),
                        rhs=src[:, MARGIN + c0 - SOLO_D:
                                MARGIN + c0 - SOLO_D + TILE],
                        start=False, stop=True)
                fuse(j, ps)

            def tanh_drain(j, ps):
                t0 = j * TILE
                nc.scalar.activation(tanh16[:, t0:t0 + TILE], ps,
                                     AF.Tanh, bias=bias[0])

            nxt_q = [0]

            def flush_outputs(jj):
                while (nxt_q[0] < NQ
                       and QEDGE[nxt_q[0] + 1] <= (jj + 1) * TILE):
                    q = nxt_q[0]
                    nc.sync.dma_start(
                        out=outs[q], in_=out16[:, QEDGE[q]:QEDGE[q + 1]])
                    nxt_q[0] += 1

            def epilogue(j, ps):
                t0 = j * TILE
                sig = epool.tile([P, TILE], fp16, name=f"sig{j}", tag="sig")
                nc.scalar.activation(sig, ps, AF.Sigmoid, bias=bias[1])
                # alternate the gating mul DVE/GpSimd: the tail is aux-bound
                eng = nc.vector if j % 2 == 0 else nc.gpsimd
                eng.tensor_mul(out16[:, t0:t0 + TILE],
                               tanh16[:, t0:t0 + TILE], sig)
                flush_outputs(j)

            # ---- startup: f-pair0 + g-pair0 share D2(x) -------------------
            wb_f0 = WBASE[("pair", "f", 0)]
            wb_g0 = WBASE[("pair", "g", 0)]
            wb_f1 = WBASE[("pair", "f", 1)]
            for s in range(NST):
                emit_D(D0, x0, 0, s)
                ppf = pair_pq("f", 0, s, x0, bufA, D0, wb_f0)
                ppg = pair_pq("g", 0, s, x0, scratch, D0, wb_g0)
                pair_r("f", 0, s, bufA, D0, wb_f0, ppf)
                pair_r("g", 0, s, scratch, D0, wb_g0, ppg)
                emit_D(D1, bufA, 1, s)       # f-pair1's D

            # ---- main chain ----------------------------------------------
            # entry list after startup: (kind, br, k, src, dst, db)
            fchain = [bufA, bufB, bufA, bufB, bufA, bufB]
            gchain = [scratch, bufA, bufB, bufA, bufB, bufA]
            plan = []
            for k in range(1, 6):
                plan.append(("pair", "f", k, fchain[k - 1], fchain[k]))
            plan.append(("solo", "f", None, fchain[5], None))
            for k in range(1, 6):
                plan.append(("pair", "g", k, gchain[k - 1], gchain[k]))
            plan.append(("solo", "g", None, gchain[5], None))

            # D-buffer per pair entry: f1 used D1 (emitted above); alternate
            dbuf_of = {}
            nd = 1
            for e in plan:
                if e[0] == "pair":
                    dbuf_of[(e[1], e[2])] = (D1, D0)[nd % 2 == 0]
                    nd += 1
            # D prefetch: only the immediately-next plan entry (if a pair)
            def next_pair(i):
                if i + 1 < len(plan) and plan[i + 1][0] == "pair":
                    return plan[i + 1]
                return None

            for i, e in enumerate(plan):
                kind, br, k, src, dst = e
                npair = next_pair(i)
                # next pair's D source is its own src buffer
                if kind == "pair":
                    db = dbuf_of[(br, k)]
                    wb = WBASE[("pair", br, k)]
                    if k == 5:
                        p5live = {}
                        if br == "g":
                            # tail: interleave solo-g + epilogue per tile
                            wbs = WBASE[("solo", "g", None)]
                            for j in range(NT):
                                pair5_tile(br, j, src, dst, db, wb, p5live)
                                if j % 2 == 1 and npair is not None:
                                    emit_D(dbuf_of[(npair[1], npair[2])],
                                           npair[3], npair[2], j // 2)
                                if j >= 2:
                                    solo_tile("g", j - 2, dst, wbs, epilogue)
                            for jj in (NT - 2, NT - 1):
                                solo_tile("g", jj, dst, wbs, epilogue)
                        else:
                            for j in range(NT):
                                pair5_tile(br, j, src, dst, db, wb, p5live)
                                if j % 2 == 1 and npair is not None:
                                    emit_D(dbuf_of[(npair[1], npair[2])],
                                           npair[3], npair[2], j // 2)
                    else:
                        # steady state: 1-supertile lag between p/q and r
                        prev = None
                        for s in range(NST):
                            pp = pair_pq(br, k, s, src, dst, db, wb)
                            if prev is not None:
                                pair_r(br, k, s - 1, dst, db, wb, prev)
                                if npair is not None:
                                    emit_D(dbuf_of[(npair[1], npair[2])],
                                           npair[3], npair[2], s - 1)
                            prev = pp
                        pair_r(br, k, NST - 1, dst, db, wb, prev)
                        if npair is not None:
                            emit_D(dbuf_of[(npair[1], npair[2])],
                                   npair[3], npair[2], NST - 1)
                elif kind == "solo" and br == "f":
                    wb = WBASE[("solo", "f", None)]
                    for j in range(NT):
                        solo_tile("f", j, src, wb, tanh_drain)
                        if j % 2 == 1 and npair is not None:
                            emit_D(dbuf_of[(npair[1], npair[2])],
                                   npair[3], npair[2], j // 2)
                # solo-g handled inside the g-pair5 tail above

    nc.compile()
    return nc


def get_program():
    if "nc" not in _PROGRAM_CACHE:
        _PROGRAM_CACHE["nc"] = _build_program()
    return _PROGRAM_CACHE["nc"]


def make_in_maps(x, h, Wf, Wg, Vf, Vg):
    x = np.asarray(x, dtype=np.float32)
    h = np.asarray(h, dtype=np.float32)
    eye2 = np.eye(2, dtype=np.float64)
    Wn = {"f": np.asarray(Wf, dtype=np.float64),
          "g": np.asarray(Wg, dtype=np.float64)}
    wpack = np.zeros((NW, P, P), dtype=np.float32)
    for (kind, br, k), base in WBASE.items():
        Wb = Wn[br]
        if kind == "pair":
            W0a, W1a = Wb[2 * k, 0], Wb[2 * k, 1]
            W0b, W1b = Wb[2 * k + 1, 0], Wb[2 * k + 1, 1]
            V0 = W1a @ W1b
            V1 = W0a @ W1b
            V2 = W1a @ W0b
            V3 = W0a @ W0b
            mats = [V0 + V2, V1 + V3, -V2, -V3, V0, V1]
        else:
            mats = [Wb[12, 1], Wb[12, 0]]
        for i, m in enumerate(mats):
            wpack[base + i] = np.kron(eye2, m).astype(np.float32)
    wcols = wpack.transpose(1, 0, 2).reshape(P, NW * P).astype(np.float16)
    vcat = np.concatenate(
        [np.kron(np.eye(2, dtype=np.float32), np.asarray(V, dtype=np.float32))
         for V in (Vf, Vg)], axis=1)  # [128, 256]

    in_maps = []
    for core in range(N_CORES):
        sl = slice(core * BPC, (core + 1) * BPC)
        xcm = x[sl].transpose(0, 2, 1).reshape(P, T) \
            .astype(np.float16)  # [(b,c), t]
        consts = np.ascontiguousarray(
            np.concatenate([vcat, h[sl].reshape(P, 1)], axis=1))
        in_maps.append({"consts": consts, "xr": xcm, "wr": wcols})
    return in_maps


def _to_f32(a):
    a = np.asarray(a)
    if a.dtype in (np.float32, np.float16):
        return a.astype(np.float32)
    u = a.view(np.uint16).astype(np.uint32) << np.uint32(16)
    return u.view(np.float32)


def assemble_output(results):
    full = np.empty((B, T, C), dtype=np.float32)
    for core, r in enumerate(results):
        cm = np.concatenate(
            [_to_f32(r[f"out_q{q}"]) for q in range(NQ)], axis=1)
        full[core * BPC:(core + 1) * BPC] = \
            cm.reshape(BPC, C, T).transpose(0, 2, 1)
    return full


def kernel(x, h, Wf, Wg, Vf, Vg):
    from concourse import bass_utils

    nc = get_program()
    in_maps = make_in_maps(x, h, Wf, Wg, Vf, Vg)
    res = bass_utils.run_bass_kernel_spmd(nc, in_maps,
                                          core_ids=list(range(N_CORES)))
    return assemble_output(res.results)
